# revision 1
# baseline (speedup 1.0000x reference)
"""Joint bilateral filter (3x3, reflect pad) on 8 trn2 cores.

Sharding: 1024 output rows (2 batches x 512 H) split as 8 x 128 rows.
Host pre-pads H and W with reflect (radius 1), so each core gets a
halo-inclusive channel-major shard and computes its [128, C, 512]
output slab with no boundary handling on device.

Device layout: partition p = output row p of the shard. Free dim is
channel-major [C, W] so the per-pixel bilateral weight (one per W pos)
broadcasts across channels via a stride-0 AP, and dx shifts are free-dim
offsets. dy shifts are handled by loading 3 row-shifted copies of the
inputs (dy = 0,1,2 -> padded rows [dy, dy+128)).

Engine split (final), per column chunk:
- DVE:  guide-difference subs and 6 of the 8 non-center tap products in
  bf16 2x packed mode (k=7 dx=1 reads a host-prepared one-element-
  shifted copy for 4B alignment), reciprocal of den, final num*(1/den).
- Act:  square + exp for the 7 computed weight fields, the w5 shift
  copy (below), and the PSUM->SBUF copies of num.
- Pool: channel-sum of the squared guide distances, den accumulation,
  and the k=1 dx=1 tap product (Pool has no packed-alignment modes, so
  it reads the odd offset directly). The k=1 product is emitted at the
  top of each iteration so it lands before PE wants it (it is PE's
  last accumulate).
- PE:   sums the 8 tap products + the center src into PSUM fp32 via
  identity-weight matmuls (psum += I @ prod) - the adds cost the
  otherwise-idle tensor engine ~1.1us per tap instead of 1.5us of DVE,
  and the fp32 PSUM accumulation improves accuracy. A filler matmul
  between taps keeps the PE busy through prod waits: its DVFS reaches
  2.4GHz only after 3us of gap-free execution and any bubble resets
  it. num is split into two half-width PSUM tiles so the next chunk's
  PE work only waits on the matching half's PSUM->SBUF copy.
- tap symmetry: w5[p,x] = w3[p,x+1] exactly (opposite taps share their
  guide-difference field; the dy=1 pair needs no row shift), so tap
  5's weight pipeline is replaced by computing w3 one column wider and
  one tiny SBUF->SBUF shift-DMA. The row-shifted pairs (w6/w7/w8)
  cannot use this: the BIR verifier rejects cross-partition SBUF DMAs.
- weights(ci+1) is emitted before mac(ci) (software pipelining), so
  the weight pipeline always has a chunk of lead time; each chunk's
  finalize (reciprocal etc) is emitted one chunk late so the in-order
  DVE stream never stalls on den/num completion.
- src arrives as 4 column-chunks per row-shifted copy, pre-chunked in
  DRAM by the host so each slab DMA is one contiguous run per
  partition and chunk-0 compute starts ~4us in.
- center tap weight is exactly 1: PE accumulates src directly, +1 via
  den's memset.
"""

import sys

sys.path.insert(0, "/opt/trn_rl_repo")

import ml_dtypes
import numpy as np

BF16 = ml_dtypes.bfloat16

B, H, W = 2, 512, 512
CS, CI = 21, 3
N_CORES = 8
ROWS = (B * H) // N_CORES  # 128 output rows per core
WP = W + 2  # padded width
CHUNK = 128  # output cols per compute chunk
N_CHUNKS = W // CHUNK
SLAB = CHUNK + 2  # chunk + dx halo

SIGMA_BILATERAL = 0.25
INV2SIG2 = 1.0 / (2.0 * SIGMA_BILATERAL**2)  # 8.0

NONCENTER = [0, 1, 2, 3, 5, 6, 7, 8]

_CACHE = {}


def _build():
    from concourse.bacc import Bacc
    from concourse.tile import TileContext
    import concourse.mybir as mybir

    fp32 = mybir.dt.float32
    bf16 = mybir.dt.bfloat16
    Alu = mybir.AluOpType
    Act = mybir.ActivationFunctionType

    nc = Bacc("TRN2", target_bir_lowering=False, debug=False, num_devices=N_CORES)
    # se: x = w0 + [0, 130); so: dy in (0, 2) rows, x = w0 + 1 + [0, 130)
    se_d = nc.dram_tensor(
        "src_e", [N_CHUNKS, ROWS + 2, CS, SLAB], bf16, kind="ExternalInput"
    )
    so_d = nc.dram_tensor(
        "src_o2", [N_CHUNKS, ROWS, CS, SLAB], bf16, kind="ExternalInput"
    )
    im_d = nc.dram_tensor("im", [ROWS + 2, CI, WP], bf16, kind="ExternalInput")
    # guide image shifted left one column (col j = im col j+1): keeps every
    # weight-pipeline sub operand 4B-aligned so DVE stays in 2x packed mode
    # on hardware (the center slice w0+1 and dx=1 minuends are odd in im)
    imo_d = nc.dram_tensor("im_o", [ROWS + 2, CI, WP], bf16, kind="ExternalInput")
    id_d = nc.dram_tensor("ident", [ROWS, ROWS], bf16, kind="ExternalInput")
    out_d = nc.dram_tensor("out", [ROWS, CS, W], bf16, kind="ExternalOutput")

    # num PSUM half-windows: channel ranges of <= 8 channels
    # (8*64 = 512 fp32, one 2KB bank per window)
    WIN2 = [(c0, min(CS, c0 + 8)) for c0 in range(0, CS, 8)]

    with TileContext(nc) as tc:
        with tc.tile_pool(name="p", bufs=1) as pool, tc.tile_pool(
            name="ps", bufs=1, space="PSUM"
        ) as ppool:
            bias_t = {}
            for v in (0.0, -0.5, -1.0):
                bt = pool.tile([ROWS, 1], fp32, tag=f"b{v}")
                nc.gpsimd.memset(bt[:], v)
                bias_t[v] = bt

            # im + identity first (guide center i_t[1] gates everything),
            # then chunk-0 src slabs, then the rest; the SP DMA queue
            # serializes in this order.
            i_t = [None] * 3
            i_to = [None] * 3
            for dy, shifted in ((1, True), (1, False), (0, False), (0, True)):
                src = imo_d if shifted else im_d
                tag = f"io{dy}" if shifted else f"i{dy}"
                it = pool.tile([ROWS, CI, WP], bf16, tag=tag)
                nc.sync.dma_start(
                    it[:].rearrange("p c w -> p (c w)"),
                    src[dy : dy + ROWS].rearrange("p c w -> p (c w)"),
                )
                (i_to if shifted else i_t)[dy] = it
            ident = pool.tile([ROWS, ROWS], bf16, tag="ident")
            nc.sync.dma_start(ident[:], id_d[:])

            s_e = [[None] * 3 for _ in range(N_CHUNKS)]
            s_o = [None] * N_CHUNKS  # dy = 2 only (k=7 on DVE needs align)

            def issue_slab(ci):
                for dy in range(3):
                    se = pool.tile([ROWS, CS, SLAB], bf16, tag=f"se{ci}_{dy}")
                    nc.sync.dma_start(
                        se[:].rearrange("p c w -> p (c w)"),
                        se_d[ci, dy : dy + ROWS].rearrange("p c w -> p (c w)"),
                    )
                    s_e[ci][dy] = se

            def issue_so(ci):
                # odd-shifted copy for the k=7 dx=1 tap
                so = pool.tile([ROWS, CS, SLAB], bf16, tag=f"so{ci}")
                nc.sync.dma_start(
                    so[:].rearrange("p c w -> p (c w)"),
                    so_d[ci].rearrange("p c w -> p (c w)"),
                )
                s_o[ci] = so

            for shifted in (False, True):
                src = imo_d if shifted else im_d
                it = pool.tile([ROWS, CI, WP], bf16, tag="io2" if shifted else "i2")
                nc.sync.dma_start(
                    it[:].rearrange("p c w -> p (c w)"),
                    src[2 : 2 + ROWS].rearrange("p c w -> p (c w)"),
                )
                (i_to if shifted else i_t)[2] = it
            for ci in range(N_CHUNKS):
                issue_slab(ci)
                issue_so(ci)

            # Each consuming engine observes every input DMA once (tiny
            # absorber ops) so real consumers don't pile up sync waits.
            # DVE consumes src, Pool consumes im.
            dummV = pool.tile([1, 1, 1], bf16, tag="dummV")

            def absorb_src_slab(ci):
                for t in s_e[ci] + [s_o[ci]]:
                    nc.vector.tensor_scalar(
                        dummV[:], t[0:1, 0:1, 0:1], 0.0, None, Alu.add
                    )

            dummP = pool.tile([1, 1, 1], bf16, tag="dummP")
            for t in i_t + i_to:
                nc.vector.tensor_scalar(dummV[:], t[0:1, 0:1, 0:1], 0.0, None, Alu.add)

            # --- software-pipelined chunk loop ---------------------------
            # weights(ci) runs one chunk ahead of mac(ci): the DVE emits
            # subs(ci+1) before mults(ci), so the Pool/Act weight pipeline
            # always has a full chunk of lead time and never starves the
            # MAC engines.
            wk_all = [None] * N_CHUNKS
            den_all = [None] * N_CHUNKS
            pp1_all = [None] * N_CHUNKS

            def pp1(ci):
                # k=1 tap product on Pool (no packed-alignment modes, reads
                # the odd offset directly); emitted at the TOP of the
                # iteration so Pool delivers it before PE needs it
                nc.gpsimd.tensor_scalar(
                    dummP[:], s_e[ci][0][0:1, 0:1, 0:1], 0.0, None, Alu.add
                )
                wk_b = wk_all[ci][1][:, 0:CHUNK].rearrange(
                    "p (x w) -> p x w", x=1
                ).broadcast_to([ROWS, CS, CHUNK])
                pt = pool.tile([ROWS, CS, CHUNK], bf16, tag="prod1")
                nc.gpsimd.tensor_tensor(
                    pt[:], s_e[ci][0][:, :, 1 : 1 + CHUNK], wk_b, Alu.mult
                )
                pp1_all[ci] = pt

            def weights(ci):
                w0 = ci * CHUNK
                den = pool.tile([ROWS, CHUNK], fp32, tag=f"den{ci}")
                nc.gpsimd.memset(den[:], 1.0)
                den_all[ci] = den
                wk = {}
                # k=5 is never computed: w5[p,x] = w3[p,x+1] (opposite taps
                # share their guide-difference field and the dy=1 pair needs
                # no row shift), so k=3 is computed one column wider and k=5
                # is a one-column shift-copy of it.
                for k in (3, 0, 1, 2, 6, 7, 8):
                    dy, dx = k // 3, k % 3
                    cw = CHUNK + 1 if k == 3 else CHUNK
                    ic = i_to[1][:, :, w0 : w0 + cw]
                    lnw1 = -0.5 * ((dx - 1) ** 2 + (dy - 1) ** 2)
                    dtag = f"d3w{ci % 3}" if k == 3 else f"d{ci % 3}_{k % 2}"
                    # row stride padded even so every channel row stays
                    # 4B-aligned for the DVE's 2x packed mode
                    cwp = cw + (cw % 2)
                    d = pool.tile([ROWS, CI, cwp], bf16, tag=dtag)
                    mn = (
                        i_to[dy][:, :, w0 : w0 + cw]
                        if dx == 1
                        else i_t[dy][:, :, w0 + dx : w0 + dx + cw]
                    )
                    nc.vector.tensor_tensor(d[:, :, 0:cw], mn, ic, Alu.subtract)
                    d2tag = f"d23w{ci % 3}" if k == 3 else f"d2{ci % 3}_{k % 2}"
                    d2 = pool.tile([ROWS, CI, cw], fp32, tag=d2tag)
                    nc.scalar.square(d2[:], d[:, :, 0:cw])
                    # sum over the 3 guide channels (Pool has no free-dim
                    # reduce, so two explicit adds)
                    wrtag = f"wr3w{ci % 3}" if k == 3 else f"wr{ci % 3}_{k % 2}"
                    wr = pool.tile([ROWS, cw], fp32, tag=wrtag)
                    nc.gpsimd.tensor_tensor(
                        wr[:], d2[:, 0, :], d2[:, 1, :], Alu.add
                    )
                    nc.gpsimd.tensor_tensor(wr[:], d2[:, 2, :], wr[:], Alu.add)
                    wt = pool.tile([ROWS, cw], bf16, tag=f"wk{ci}_{k}")
                    nc.scalar.activation(
                        wt[:], wr[:], Act.Exp, bias=bias_t[lnw1][:], scale=-INV2SIG2
                    )
                    wk[k] = wt
                # w5 <- w3 shifted: issued on the Act DGE queue right after
                # exp(k=3), so same-engine program order replaces a sem wait
                w5 = pool.tile([ROWS, CHUNK], bf16, tag=f"wk{ci}_5")
                nc.scalar.dma_start(w5[:], wk[3][:, 1 : 1 + CHUNK])
                wk[5] = w5
                wk_all[ci] = wk

            def mac(ci):
                w0 = ci * CHUNK
                wk = wk_all[ci]
                # tap products: k=1 on Pool (it has slack), rest on DVE in
                # bf16 2x packed mode; dx=1 taps last (their shifted copies
                # arrive after the main slab).
                absorb_src_slab(ci)
                nc.vector.tensor_scalar(
                    dummV[:], wk[5][0:1, 0:1], 0.0, None, Alu.add
                )
                nc.gpsimd.tensor_scalar(
                    dummP[:], wk[5][0:1, 0:1], 0.0, None, Alu.add
                )
                den = den_all[ci]
                for k in (0, 1, 2, 3, 6, 7, 8, 5):
                    nc.gpsimd.tensor_tensor(
                        den[:], wk[k][:, 0:CHUNK], den[:], Alu.add
                    )
                prods = {}
                prods[1] = pp1_all[ci]
                for k in (0, 2, 3, 6, 8, 5, 7):
                    dy, dx = k // 3, k % 3
                    wk_b = wk[k][:, 0:CHUNK].rearrange(
                        "p (x w) -> p x w", x=1
                    ).broadcast_to([ROWS, CS, CHUNK])
                    if k == 7:
                        sk = s_o[ci][:, :, 0:CHUNK]
                    else:
                        sk = s_e[ci][dy][:, :, dx : dx + CHUNK]
                    pt = pool.tile([ROWS, CS, CHUNK], bf16, tag=f"prod{k}")
                    nc.vector.tensor_tensor(pt[:], sk, wk_b, Alu.mult)
                    prods[k] = pt

                # PE: num = sum of products + center src (read directly, PE
                # has no alignment modes), accumulated in PSUM fp32 via
                # identity matmuls. Center first (always ready); a filler
                # matmul between taps keeps the PE busy through prod waits
                # (PE DVFS reaches 2.4GHz only after 3us gap-free). num is
                # split into two half-width PSUM tiles so the next chunk's
                # PE work only waits on the matching half's PSUM->SBUF copy.
                half = CHUNK // 2
                numps = []
                for h in range(2):
                    np_h = ppool.tile([ROWS, CS, half], fp32, tag=f"nump{h}")
                    numps.append(np_h)
                fill = ppool.tile([ROWS, 512], fp32, tag="fill")

                def filler(n):
                    for _ in range(n):
                        nc.tensor.matmul(
                            fill[:], ident[:],
                            s_e[ci][0][:, 0:4, 0:CHUNK],
                            start=True, stop=True,
                        )

                acc_srcs = [s_e[ci][1][:, :, 1 : 1 + CHUNK]]
                acc_srcs += [prods[k][:] for k in (0, 2, 3, 5, 6, 7, 8, 1)]
                n_acc = len(acc_srcs)
                for t, ap in enumerate(acc_srcs):
                    for h in range(2):
                        for a, b in WIN2:
                            nc.tensor.matmul(
                                numps[h][:, a:b, :],
                                ident[:],
                                ap[:, a:b, h * half : (h + 1) * half],
                                start=(t == 0), stop=(t == n_acc - 1),
                            )
                    if t < n_acc - 1:
                        filler(1)

                # Act: PSUM -> SBUF (bf16) per-half copies of num
                numb = pool.tile([ROWS, CS, CHUNK], bf16, tag="numb", bufs=2)
                for h in range(2):
                    nc.scalar.copy(
                        numb[:, :, h * half : (h + 1) * half], numps[h][:]
                    )

                def finalize():
                    den = den_all[ci]
                    rd = pool.tile([ROWS, CHUNK], fp32, tag=f"rd{ci}")
                    nc.vector.reciprocal(rd[:], den[:])
                    rdb = pool.tile([ROWS, CHUNK], bf16, tag=f"rdb{ci}")
                    nc.vector.tensor_scalar(rdb[:], rd[:], 0.0, None, Alu.add)
                    outt = pool.tile([ROWS, CS, CHUNK], bf16, tag="outt", bufs=2)
                    # last chunk: two half-width pieces so the final output
                    # DMA overlaps the second outmult instead of trailing it
                    parts = (
                        [(0, CHUNK // 2), (CHUNK // 2, CHUNK)]
                        if ci == N_CHUNKS - 1
                        else [(0, CHUNK)]
                    )
                    for a, b in parts:
                        rdb_b = rdb[:, a:b].rearrange(
                            "p (x w) -> p x w", x=1
                        ).broadcast_to([ROWS, CS, b - a])
                        nc.vector.tensor_tensor(
                            outt[:, :, a:b], numb[:, :, a:b], rdb_b, Alu.mult
                        )
                        nc.sync.dma_start(
                            out_d[:, :, w0 + a : w0 + b], outt[:, :, a:b]
                        )

                return finalize

            pending_finalize = None
            weights(0)
            for ci in range(N_CHUNKS):
                pp1(ci)
                if ci + 1 < N_CHUNKS:
                    weights(ci + 1)
                fin = mac(ci)
                if pending_finalize is not None:
                    pending_finalize()
                pending_finalize = fin
            pending_finalize()
    nc.compile()
    return nc


def _get_nc():
    if "nc" not in _CACHE:
        _CACHE["nc"] = _build()
    return _CACHE["nc"]


def _shard_inputs(src, im):
    srcp = np.pad(src, ((0, 0), (1, 1), (1, 1), (0, 0)), mode="reflect")
    imp = np.pad(im, ((0, 0), (1, 1), (1, 1), (0, 0)), mode="reflect")
    # channel-major: [B, Hp, C, Wp], bf16; pad 2 junk cols so the odd-shift
    # slab slices below stay in range
    srcp = np.transpose(srcp, (0, 1, 3, 2)).astype(BF16)
    srcp = np.pad(srcp, ((0, 0), (0, 0), (0, 0), (0, 2)))
    imp = np.ascontiguousarray(np.transpose(imp, (0, 1, 3, 2))).astype(BF16)
    impo = np.pad(imp[:, :, :, 1:], ((0, 0), (0, 0), (0, 0), (0, 1)))
    ident = np.eye(ROWS, dtype=BF16)
    in_maps = []
    for core in range(N_CORES):
        b, r0 = core // 4, (core % 4) * ROWS
        sl = srcp[b, r0 : r0 + ROWS + 2]  # [130, 21, 516]
        se = np.stack(
            [sl[:, :, ci * CHUNK : ci * CHUNK + SLAB] for ci in range(N_CHUNKS)]
        )
        so = np.stack(
            [
                sl[2 : 2 + ROWS, :, ci * CHUNK + 1 : ci * CHUNK + 1 + SLAB]
                for ci in range(N_CHUNKS)
            ]
        )
        in_maps.append(
            {
                "src_e": np.ascontiguousarray(se),
                "src_o2": np.ascontiguousarray(so),
                "im": np.ascontiguousarray(imp[b, r0 : r0 + ROWS + 2]),
                "im_o": np.ascontiguousarray(impo[b, r0 : r0 + ROWS + 2]),
                "ident": ident,
            }
        )
    return in_maps


def kernel(src, im, _trace=False, _tmpdir=None):
    from concourse import bass_utils

    src = np.asarray(src, dtype=np.float32)
    im = np.asarray(im, dtype=np.float32)
    nc = _get_nc()
    in_maps = _shard_inputs(src, im)
    res = bass_utils.run_bass_kernel_spmd(
        nc, in_maps, core_ids=list(range(N_CORES)), trace=_trace, tmpdir=_tmpdir
    )
    out = np.empty((B, H, W, CS), dtype=np.float32)
    for core in range(N_CORES):
        b, r0 = core // 4, (core % 4) * ROWS
        o = res.results[core]["out"]  # [128, 21, 512] bf16
        out[b, r0 : r0 + ROWS] = np.transpose(o, (0, 2, 1)).astype(np.float32)
    _CACHE["last_results"] = res
    return out



# revision 12
# speedup vs baseline: 1.1564x; 1.1564x over previous
"""Joint bilateral filter (3x3, reflect pad) on 8 trn2 cores.

Sharding: 1024 output rows (2 batches x 512 H) split as 8 x 128 rows.
Host pre-pads H and W with reflect (radius 1), so each core gets a
halo-inclusive channel-major shard and computes its [128, C, 512]
output slab with no boundary handling on device.

Device layout: partition p = output row p of the shard. Free dim is
channel-major [C, W] so the per-pixel bilateral weight (one per W pos)
broadcasts across channels via a stride-0 AP, and dx shifts are free-dim
offsets. dy shifts are handled by loading 3 row-shifted copies of the
inputs (dy = 0,1,2 -> padded rows [dy, dy+128)).

Key structure, per column chunk:
- The 3x3 spatial kernel is folded into PE's weight matrices: the host
  sends three scaled identities w1*I (w1 = exp(-s/2), s = squared tap
  distance in {0,1,2}) and each tap's PSUM accumulate uses the identity
  matching its spatial weight. The guide weights wk are therefore raw
  exp(-8*||guide diff||^2): one batched square, two batched channel-sum
  adds, and ONE batched exp with shared scale for all 7 fields.
- den = 1 + sum_k w1_k*wk_k is ALSO accumulated on PE: eight 128-free
  matmuls (w1*I @ wk_slot) into a 1-bank PSUM tile, +1 via an Act
  Copy-with-bias into SBUF. Pool's only remaining weight work is the
  two batched channel-sum adds.
- DVE: guide-difference subs (one op per dy group covering its three
  dx shifts via a hand-built overlapping 3-dim AP), 6.4 of the 8
  non-center tap products in bf16 2x packed mode, reciprocal of den,
  and the final num*(1/den).
- Pool: the k=1 tap product and most of the k=7 product (its column
  split balances DVE vs Pool), plus the channel sums.
- PE:   num = sum of 8 tap products + center src via identity-weight
  matmuls into two half-width PSUM tiles (the adds cost the
  otherwise-idle tensor engine ~1.1us per tap instead of 1.5us of
  DVE, and fp32 PSUM accumulation improves accuracy). A narrow filler
  matmul between taps keeps the PE's DVFS ramped through prod waits
  (2.4GHz needs 3us of gap-free execution; any bubble resets it).
- tap symmetry: w5[p,x] = w3[p,x+1] exactly (opposite taps share their
  guide-difference field; the dy=1 pair needs no row shift), so tap
  5's weight is an offset view into k=3's slot of the batched weight
  tile. The row-shifted pairs cannot use this: their shift crosses
  partitions.
- weights(ci+1) is emitted before mac(ci) (software pipelining), so
  the weight pipeline always has a chunk of lead time; each chunk's
  finalize (reciprocal etc) is emitted one chunk late so the in-order
  DVE stream never stalls on den/num completion.
- src arrives as 4 column-chunks per row-shifted copy, pre-chunked in
  DRAM by the host so each slab DMA is one contiguous run per
  partition; the guide arrives as small per-chunk row-shifted tiles,
  interleaved into the DMA queue so chunk-0 weights start ~1.5us in.
  The output DRAM tensor is chunked the same way so each out-DMA
  descriptor is one 5.4KB run per partition (runs under 512B pay 2x
  in the DMA engines). The last chunk's finalize is split in half so
  the final output DMAs overlap the tail of the compute.
- center tap weight is exactly 1: PE accumulates src directly, +1 for
  den via the Act bias.
"""

import sys

sys.path.insert(0, "/opt/trn_rl_repo")

import math

import ml_dtypes
import numpy as np

BF16 = ml_dtypes.bfloat16

B, H, W = 2, 512, 512
CS, CI = 21, 3
N_CORES = 8
ROWS = (B * H) // N_CORES  # 128 output rows per core
WP = W + 2  # padded width
CHUNK = 128  # output cols per compute chunk
N_CHUNKS = W // CHUNK
SLAB = CHUNK + 2  # chunk + dx halo
IMW = CHUNK + 4  # chunk + dx halo + k3/w5 extra col
CW = CHUNK + 2  # computed weight-field width (k3 needs +1 col for the w5 view)
PSPLIT = 80  # k=7 product: cols [0,PSPLIT) on Pool, rest on DVE

SIGMA_BILATERAL = 0.25
INV2SIG2 = 1.0 / (2.0 * SIGMA_BILATERAL**2)  # 8.0

# weight-field slots in the batched D/WR/WK tiles:
#   slot 0..2 = taps (dy=0, dx=0..2) = k0,k1,k2
#   slot 3..5 = taps (dy=2, dx=0..2) = k6,k7,k8
#   slot 6    = tap  (dy=1, dx=0)    = k3   (k5 = slot 6, one col right)
SLOT_OF = {0: 0, 1: 1, 2: 2, 6: 3, 7: 4, 8: 5, 3: 6}
# spatial-kernel index per tap: (dx-1)^2 + (dy-1)^2 in {0,1,2}
SIDX = {k: ((k % 3) - 1) ** 2 + ((k // 3) - 1) ** 2 for k in range(9)}

_CACHE = {}


def _ap_with(ap, dims):
    """Copy `ap` and replace its free dims (keeps partition dim + offset)."""
    import bass_rust

    c = ap.copy()
    part = list(c.ap)[0]
    c.ap = bass_rust.VecI64Pair([list(part)] + [list(d) for d in dims])
    return c


def _build():
    from concourse.bacc import Bacc
    from concourse.tile import TileContext
    import concourse.mybir as mybir

    fp32 = mybir.dt.float32
    bf16 = mybir.dt.bfloat16
    Alu = mybir.AluOpType
    Act = mybir.ActivationFunctionType

    nc = Bacc("TRN2", target_bir_lowering=False, debug=False, num_devices=N_CORES)
    se_d = nc.dram_tensor(
        "src_e", [N_CHUNKS, ROWS + 2, CS, SLAB], bf16, kind="ExternalInput"
    )
    # per-chunk row-shifted guide tiles: [chunk, dy, row, c, x]
    im_d = nc.dram_tensor("im", [N_CHUNKS, 3, ROWS, CI, IMW], bf16, kind="ExternalInput")
    # three spatially-scaled identities: exp(-s/2) * I for s = 0, 1, 2
    id_d = nc.dram_tensor("ident", [3, ROWS, ROWS], bf16, kind="ExternalInput")
    out_d = nc.dram_tensor(
        "out", [N_CHUNKS, ROWS, CS, CHUNK], bf16, kind="ExternalOutput"
    )

    # num PSUM half-windows: channel ranges of <= 8 channels
    # (8*64 = 512 fp32, one 2KB bank per window)
    WIN2 = [(c0, min(CS, c0 + 8)) for c0 in range(0, CS, 8)]

    with TileContext(nc) as tc:
        with tc.tile_pool(name="p", bufs=1) as pool, tc.tile_pool(
            name="ps", bufs=1, space="PSUM"
        ) as ppool:
            # ---- input DMAs, interleaved for a fast chunk-0 start -------
            imt = [None] * N_CHUNKS
            s_e = [[None] * 3 for _ in range(N_CHUNKS)]

            def issue_im(ci):
                it = pool.tile([ROWS, 3, CI, IMW], bf16, tag=f"im{ci}")
                for dy in range(3):
                    nc.sync.dma_start(
                        it[:, dy].rearrange("p c w -> p (c w)"),
                        im_d[ci, dy].rearrange("p c w -> p (c w)"),
                    )
                imt[ci] = it

            def issue_slab(ci, dys):
                for dy in dys:
                    se = pool.tile([ROWS, CS, SLAB], bf16, tag=f"se{ci}_{dy}")
                    nc.sync.dma_start(
                        se[:].rearrange("p c w -> p (c w)"),
                        se_d[ci, dy : dy + ROWS].rearrange("p c w -> p (c w)"),
                    )
                    s_e[ci][dy] = se

            issue_im(0)
            issue_slab(0, (0,))
            issue_im(1)
            ident = pool.tile([ROWS, 3, ROWS], bf16, tag="ident")
            for s in range(3):
                nc.sync.dma_start(ident[:, s], id_d[s])
            issue_slab(0, (1, 2))
            issue_slab(1, (0, 1, 2))
            issue_im(2)
            issue_slab(2, (0, 1, 2))
            issue_im(3)
            issue_slab(3, (0, 1, 2))

            def idw(s):
                return ident[:, s]

            # Each consuming engine observes every slab DMA once (tiny
            # absorber ops) so real consumers don't pile up sync waits.
            dummV = pool.tile([1, 1, 1], bf16, tag="dummV")
            dummP = pool.tile([1, 1, 1], bf16, tag="dummP")

            def absorb_src_slab(ci):
                for t in s_e[ci]:
                    nc.vector.tensor_scalar(
                        dummV[:], t[0:1, 0:1, 0:1], 0.0, None, Alu.add
                    )

            # --- software-pipelined chunk loop ---------------------------
            wk_all = [None] * N_CHUNKS
            dens_all = [None] * N_CHUNKS
            pp1_all = [None] * N_CHUNKS
            pp7_all = [None] * N_CHUNKS

            def wk_view(ci, k, a=0, b=CHUNK):
                wk = wk_all[ci]
                s = SLOT_OF[3 if k == 5 else k]
                off = (1 if k == 5 else 0) + a
                return (
                    wk[:, s : s + 1, off : off + (b - a)]
                    .broadcast_to([ROWS, CS, b - a])
                )

            def pp1(ci):
                # k=1 tap product on Pool; emitted at the TOP of the
                # iteration so Pool delivers it before PE needs it
                nc.gpsimd.tensor_scalar(
                    dummP[:], s_e[ci][0][0:1, 0:1, 0:1], 0.0, None, Alu.add
                )
                pt = pool.tile([ROWS, CS, CHUNK], bf16, tag="prod1")
                nc.gpsimd.tensor_tensor(
                    pt[:], s_e[ci][0][:, :, 1 : 1 + CHUNK], wk_view(ci, 1), Alu.mult
                )
                pp1_all[ci] = pt
                # Pool's share of the k=7 product
                nc.gpsimd.tensor_scalar(
                    dummP[:], s_e[ci][2][0:1, 0:1, 0:1], 0.0, None, Alu.add
                )
                pt7 = pool.tile([ROWS, CS, CHUNK], bf16, tag="prod7")
                nc.gpsimd.tensor_tensor(
                    pt7[:, :, 0:PSPLIT],
                    s_e[ci][2][:, :, 1 : 1 + PSPLIT],
                    wk_view(ci, 7, 0, PSPLIT),
                    Alu.mult,
                )
                pp7_all[ci] = pt7

            def weights(ci):
                it = imt[ci]
                d = pool.tile([ROWS, 7, CI, CW], bf16, tag=f"d{ci % 2}")
                # the three dx-shifted fields of each row-shifted dy group
                # in ONE sub via an overlapping (dx, c, x) access pattern
                # (engine APs allow at most 3 free dims, so one op per dy)
                dcw = CI * CW
                for g, dy in enumerate((0, 2)):
                    mn3 = _ap_with(it[:, dy], [[1, 3], [IMW, 3], [1, CW]])
                    ct3 = _ap_with(it[:, 1, :, 1:], [[0, 3], [IMW, 3], [1, CW]])
                    o3 = _ap_with(d[:, 3 * g], [[dcw, 3], [CW, 3], [1, CW]])
                    nc.vector.tensor_tensor(o3, mn3, ct3, Alu.subtract)
                # k=3 field (dy=1, dx=0)
                nc.vector.tensor_tensor(
                    d[:, 6], it[:, 1, :, 0:CW], it[:, 1, :, 1 : 1 + CW], Alu.subtract
                )
                # one batched square of all 7 fields
                d2 = pool.tile([ROWS, 7, CI, CW], fp32, tag=f"d2{ci % 2}")
                nc.scalar.square(d2[:], d[:])
                # channel sums on Pool, batched across the 7 fields
                wr = pool.tile([ROWS, 7, CW], fp32, tag=f"wr{ci % 2}")
                nc.gpsimd.tensor_tensor(wr[:], d2[:, :, 0], d2[:, :, 1], Alu.add)
                nc.gpsimd.tensor_tensor(wr[:], d2[:, :, 2], wr[:], Alu.add)
                # one batched exp for all 7 fields (raw guide weights; the
                # spatial factors live in PE's scaled identities)
                wk = pool.tile([ROWS, 7, CW], bf16, tag=f"wk{ci % 2}")
                nc.scalar.activation(wk[:], wr[:], Act.Exp, bias=0.0, scale=-INV2SIG2)
                wk_all[ci] = wk

            def mac(ci):
                wk = wk_all[ci]
                absorb_src_slab(ci)
                nc.vector.tensor_scalar(
                    dummV[:], wk[0:1, 0:1, 0:1], 0.0, None, Alu.add
                )
                prods = {}
                prods[1] = pp1_all[ci]
                prods[7] = pp7_all[ci]
                # DVE products, ordered by slab arrival (dy 0, 1, 2);
                # k=7's tail columns complement Pool's share
                for k in (0, 2, 3, 5, 6, 8, 7):
                    dy, dx = k // 3, k % 3
                    if k == 7:
                        pt = prods[7]
                        nc.vector.tensor_tensor(
                            pt[:, :, PSPLIT:CHUNK],
                            s_e[ci][2][:, :, 1 + PSPLIT : 1 + CHUNK],
                            wk_view(ci, 7, PSPLIT, CHUNK),
                            Alu.mult,
                        )
                        continue
                    pt = pool.tile([ROWS, CS, CHUNK], bf16, tag=f"prod{k}")
                    nc.vector.tensor_tensor(
                        pt[:], s_e[ci][dy][:, :, dx : dx + CHUNK], wk_view(ci, k),
                        Alu.mult,
                    )
                    prods[k] = pt

                # PE part 1: den = sum_k w1_k * wk_k in a 1-bank PSUM tile
                # (eight 128-free matmuls with the scaled identities)
                denp = ppool.tile([ROWS, CHUNK], fp32, tag="denp")
                dks = (0, 1, 2, 3, 6, 7, 8, 5)
                for i, k in enumerate(dks):
                    s = SLOT_OF[3 if k == 5 else k]
                    off = 1 if k == 5 else 0
                    nc.tensor.matmul(
                        denp[:], idw(SIDX[k]), wk[:, s, off : off + CHUNK],
                        start=(i == 0), stop=(i == len(dks) - 1),
                    )
                # den -> SBUF with the center tap's +1 folded into the bias
                dens = pool.tile([ROWS, CHUNK], fp32, tag=f"dens{ci}")
                nc.scalar.activation(dens[:], denp[:], Act.Copy, bias=1.0)
                dens_all[ci] = dens

                # PE part 2: num = center + sum of tap products, in two
                # half-width PSUM tiles; narrow fillers keep DVFS ramped
                half = CHUNK // 2
                numps = []
                for h in range(2):
                    np_h = ppool.tile([ROWS, CS, half], fp32, tag=f"nump{h}")
                    numps.append(np_h)
                fill = ppool.tile([ROWS, CHUNK], fp32, tag="fill")

                def filler(n):
                    for _ in range(n):
                        nc.tensor.matmul(
                            fill[:], idw(0), s_e[ci][0][:, 0:1, 0:CHUNK],
                            start=True, stop=True,
                        )

                acc = [(0, s_e[ci][1][:, :, 1 : 1 + CHUNK])]
                acc += [(SIDX[k], prods[k][:]) for k in (0, 2, 3, 5, 6, 8, 7, 1)]
                n_acc = len(acc)
                for t, (sx, ap) in enumerate(acc):
                    for h in range(2):
                        for a, b in WIN2:
                            nc.tensor.matmul(
                                numps[h][:, a:b, :],
                                idw(sx),
                                ap[:, a:b, h * half : (h + 1) * half],
                                start=(t == 0), stop=(t == n_acc - 1),
                            )
                    if t < n_acc - 1:
                        filler(1)

                # Act: PSUM -> SBUF (bf16) per-half copies of num
                numb = pool.tile([ROWS, CS, CHUNK], bf16, tag="numb", bufs=2)
                for h in range(2):
                    nc.scalar.copy(
                        numb[:, :, h * half : (h + 1) * half], numps[h][:]
                    )

                def finalize():
                    dens = dens_all[ci]
                    rd = pool.tile([ROWS, CHUNK], fp32, tag=f"rd{ci}")
                    nc.vector.reciprocal(rd[:], dens[:])
                    rdb = pool.tile([ROWS, CHUNK], bf16, tag=f"rdb{ci}")
                    nc.vector.tensor_scalar(rdb[:], rd[:], 0.0, None, Alu.add)
                    outt = pool.tile([ROWS, CS, CHUNK], bf16, tag="outt", bufs=2)
                    # last chunk: two half-width pieces so the final output
                    # DMAs overlap the tail of the compute
                    if ci == N_CHUNKS - 1:
                        parts = [(0, CHUNK // 2), (CHUNK // 2, CHUNK)]
                    else:
                        parts = [(0, CHUNK)]
                    for a, b in parts:
                        rdb_b = rdb[:, a:b].rearrange(
                            "p (x w) -> p x w", x=1
                        ).broadcast_to([ROWS, CS, b - a])
                        nc.vector.tensor_tensor(
                            outt[:, :, a:b], numb[:, :, a:b], rdb_b, Alu.mult
                        )
                        nc.sync.dma_start(out_d[ci][:, :, a:b], outt[:, :, a:b])

                return finalize

            pending_finalize = None
            weights(0)
            for ci in range(N_CHUNKS):
                pp1(ci)
                if ci + 1 < N_CHUNKS:
                    weights(ci + 1)
                fin = mac(ci)
                if pending_finalize is not None:
                    pending_finalize()
                pending_finalize = fin
            pending_finalize()
    nc.compile()
    return nc


def _get_nc():
    if "nc" not in _CACHE:
        _CACHE["nc"] = _build()
    return _CACHE["nc"]


def _shard_inputs(src, im):
    srcp = np.pad(src, ((0, 0), (1, 1), (1, 1), (0, 0)), mode="reflect")
    imp = np.pad(im, ((0, 0), (1, 1), (1, 1), (0, 0)), mode="reflect")
    # channel-major: [B, Hp, C, Wp], bf16; pad junk cols so slab/im slices
    # below stay in range
    srcp = np.transpose(srcp, (0, 1, 3, 2)).astype(BF16)
    srcp = np.pad(srcp, ((0, 0), (0, 0), (0, 0), (0, 2)))
    imp = np.transpose(imp, (0, 1, 3, 2)).astype(BF16)
    imp = np.pad(imp, ((0, 0), (0, 0), (0, 0), (0, (N_CHUNKS - 1) * CHUNK + IMW - WP)))
    ident = np.stack(
        [(np.eye(ROWS) * math.exp(-0.5 * s)).astype(BF16) for s in range(3)]
    )
    in_maps = []
    for core in range(N_CORES):
        b, r0 = core // 4, (core % 4) * ROWS
        sl = srcp[b, r0 : r0 + ROWS + 2]  # [130, 21, 516]
        se = np.stack(
            [sl[:, :, ci * CHUNK : ci * CHUNK + SLAB] for ci in range(N_CHUNKS)]
        )
        imc = imp[b, r0 : r0 + ROWS + 2]  # [130, 3, >=516]
        imt = np.stack(
            [
                np.stack(
                    [
                        imc[dy : dy + ROWS, :, ci * CHUNK : ci * CHUNK + IMW]
                        for dy in range(3)
                    ]
                )
                for ci in range(N_CHUNKS)
            ]
        )  # [4, 3, 128, 3, IMW]
        in_maps.append(
            {
                "src_e": np.ascontiguousarray(se),
                "im": np.ascontiguousarray(imt),
                "ident": ident,
            }
        )
    return in_maps


def kernel(src, im, _trace=False, _tmpdir=None):
    from concourse import bass_utils

    src = np.asarray(src, dtype=np.float32)
    im = np.asarray(im, dtype=np.float32)
    nc = _get_nc()
    in_maps = _shard_inputs(src, im)
    res = bass_utils.run_bass_kernel_spmd(
        nc, in_maps, core_ids=list(range(N_CORES)), trace=_trace, tmpdir=_tmpdir
    )
    out = np.empty((B, H, W, CS), dtype=np.float32)
    for core in range(N_CORES):
        b, r0 = core // 4, (core % 4) * ROWS
        o = res.results[core]["out"]  # [4, 128, 21, 128] bf16
        for ci in range(N_CHUNKS):
            out[b, r0 : r0 + ROWS, ci * CHUNK : (ci + 1) * CHUNK, :] = np.transpose(
                o[ci], (0, 2, 1)
            ).astype(np.float32)
    _CACHE["last_results"] = res
    return out
